# revision 1
# baseline (speedup 1.0000x reference)
"""Trainium2 Bass kernel for an InteractionPPBlock-style GNN message-passing layer.

Strategy (8 NeuronCores, no collectives):
  * Edges are partitioned 25000/core by idx_ji ownership; triplets land on the
    core that owns their scatter destination (idx_ji), so segment_sum is local.
  * Host-side graph partitioning: each core's edges are dealt (snake order by
    degree) into 196 bins of 128 edges whose triplet counts are balanced, then
    triplets are sorted by (bin, slot) and padded so every bin holds exactly
    K*128 triplets.  This makes the device program uniform across cores (SPMD).
  * The gather table x_kjd = swish((swish(x@w_kj+b_kj)*rbf_e) @ w_down) is
    computed replicated on every core (full E) in fp16; gather is an indirect
    DMA of 128B rows.  Scatter-add is a one-hot matmul accumulated in PSUM.
  * Dense per-edge tail (x_ji, w_up, residual blocks) runs in fp32 on the
    feature-transposed layout so weights [in,out] serve directly as lhsT.
"""

import math
from contextlib import ExitStack

import numpy as np

import concourse.bass as bass
import concourse.mybir as mybir
import concourse.tile as tile
from concourse import bacc
from concourse.bass_utils import run_bass_kernel_spmd

F32 = mybir.dt.float32
F16 = mybir.dt.float16
I32 = mybir.dt.int32

HID, INT, BAS, NR, NS = 128, 64, 8, 6, 7
SR = NS * NR  # 42
P = 128


# ----------------------------------------------------------------------------
# Host-side graph partitioning (free: runs in numpy, not on device)
# ----------------------------------------------------------------------------
def _preprocess(x, rbf, sbf, idx_kj, idx_ji, n_cores):
    E = x.shape[0]
    T = sbf.shape[0]
    eper = E // n_cores                      # original edges per core
    nblk = math.ceil(eper / P)
    if nblk % 4:
        nblk += 4 - nblk % 4                 # phase-C superblocks of 4 blocks
    eperc = nblk * P                         # padded edges per core
    etot = n_cores * eperc

    idx_kj = idx_kj.astype(np.int64)
    idx_ji = idx_ji.astype(np.int64)
    owner_t = idx_ji // eper                 # triplet -> core
    deg = np.bincount(idx_ji, minlength=E)

    # snake-deal each core's edges (by degree desc) into nblk bins x 128 slots
    newlocal = np.empty(E, dtype=np.int64)
    binsum_max = 0
    for c in range(n_cores):
        lo, hi = c * eper, (c + 1) * eper
        d = deg[lo:hi]
        order = np.argsort(-d, kind="stable")
        r = np.arange(eper)
        q, pos = r // nblk, r % nblk
        bins = np.where(q % 2 == 0, pos, nblk - 1 - pos)
        nl = np.empty(eper, dtype=np.int64)
        nl[order] = bins * P + q
        newlocal[lo:hi] = nl
        bs = np.bincount(bins, weights=d[order].astype(np.float64), minlength=nblk)
        binsum_max = max(binsum_max, int(bs.max()))

    K = max(1, math.ceil(binsum_max / P))
    cap = K * P
    nchunk = nblk * K
    tpad = nblk * cap
    new_global = (np.arange(E) // eper) * eperc + newlocal

    per_core = []
    for c in range(n_cores):
        tri = np.nonzero(owner_t == c)[0]
        nlji = newlocal[idx_ji[tri]]
        b_of, s_of = nlji // P, nlji % P
        o2 = np.argsort(b_of * P + s_of, kind="stable")
        tri, b_of, s_of = tri[o2], b_of[o2], s_of[o2]
        cnt = np.bincount(b_of, minlength=nblk)
        starts = np.zeros(nblk, dtype=np.int64)
        starts[1:] = np.cumsum(cnt)[:-1]
        rank = np.arange(len(tri)) - np.repeat(starts, cnt)
        pos = b_of * cap + rank

        kj_new = np.zeros(tpad, dtype=np.int32)
        kj_new[pos] = new_global[idx_kj[tri]].astype(np.int32)
        ji_sh = np.zeros(tpad, dtype=np.float32)
        ji_sh[pos] = s_of.astype(np.float32)
        sbf_pad = np.zeros((tpad, SR), dtype=np.float16)
        sbf_pad[pos] = sbf[tri].astype(np.float16)

        idx_grid = np.ascontiguousarray(kj_new.reshape(nchunk, P).T)
        ji_grid = np.ascontiguousarray(ji_sh.reshape(nchunk, P).T)
        # 2-stack sbf^T at partition offsets {0, 64}: PE operand base
        # partitions must be in {0, 32, 64}.
        X = sbf_pad.reshape(nchunk // 2, 2, P, SR)  # [g, parity, p, k]
        sbf_t2 = np.zeros((64 + SR, (nchunk // 2) * P), dtype=np.float16)
        Xt = X.transpose(1, 3, 0, 2).reshape(2, SR, (nchunk // 2) * P)
        sbf_t2[0:SR] = Xt[0]
        sbf_t2[64:64 + SR] = Xt[1]
        per_core.append(dict(idxg=idx_grid, jig=ji_grid, sbft=sbf_t2))

    # globally renumbered x / rbf
    x_g = np.zeros((etot, HID), dtype=np.float32)
    x_g[new_global] = x
    rbf_g = np.zeros((etot, NR), dtype=np.float32)
    rbf_g[new_global] = rbf
    xt16 = np.ascontiguousarray(x_g.T.astype(np.float16))
    nsb_a = etot // 512
    # 3-stack rbf^T at partition offsets {0, 32, 64} (PE base-partition rule)
    ngrp_r = math.ceil(nsb_a / 3)
    rbf_gp = np.zeros((ngrp_r * 3 * 512, NR), dtype=np.float32)
    rbf_gp[:etot] = rbf_g
    Rt = (rbf_gp.reshape(ngrp_r, 3, 512, NR).transpose(1, 3, 0, 2)
          .reshape(3, NR, ngrp_r * 512).astype(np.float16))
    rbfp = np.zeros((64 + NR, ngrp_r * 512), dtype=np.float16)
    for g in range(3):
        rbfp[32 * g:32 * g + NR] = Rt[g]
    for c in range(n_cores):
        per_core[c]["xt32"] = np.ascontiguousarray(
            x_g[c * eperc:(c + 1) * eperc].T.astype(np.float32))

    dims = dict(n_cores=n_cores, E=E, T=T, eper=eper, nblk=nblk, eperc=eperc,
                etot=etot, K=K, cap=cap, nchunk=nchunk, nsb_a=nsb_a,
                ngrp_r=ngrp_r)
    shared = dict(xt16=xt16, rbfp=rbfp,
                  iota=np.tile(np.arange(P, dtype=np.float32), (P, 1)))
    return dims, shared, per_core, new_global


# ----------------------------------------------------------------------------
# Device program
# ----------------------------------------------------------------------------
def _build(nc, d):
    nblk, K, nchunk = d["nblk"], d["K"], d["nchunk"]
    etot, eperc, nsb_a, ngrp_r = d["etot"], d["eperc"], d["nsb_a"], d["ngrp_r"]
    nsb_c = nblk // 4

    def din(name, shape, dt):
        return nc.dram_tensor(name, shape, dt, kind="ExternalInput").ap()

    xt16 = din("xt16", [P, etot], F16)
    rbfp = din("rbfp", [64 + NR, ngrp_r * 512], F16)
    iota = din("iota", [P, P], F32)
    sbft = din("sbft", [64 + SR, (nchunk // 2) * P], F16)
    idxg = din("idxg", [P, nchunk], I32)
    jig = din("jig", [P, nchunk], F32)
    xt32 = din("xt32", [P, eperc], F32)

    wn = ["w_kj", "w_down", "w_ji", "w_up", "rb0_w1", "rb0_w2", "w_lin",
          "ra0_w1", "ra0_w2", "ra1_w1", "ra1_w2"]
    wshape = dict(w_kj=[HID, HID], w_down=[HID, INT],
                  w_ji=[HID, HID], w_up=[INT, HID])
    Wsb2 = din("Wsb2", [64 + SR, INT], F16)   # w_sbf1@w_sbf2, stacked @ {0,64}
    Wr3 = din("Wr3", [64 + NR, HID], F16)     # w_rbf1@w_rbf2, stacked @ {0,32,64}
    W = {n: din(n, wshape.get(n, [HID, HID]), F32) for n in wn}
    bn = ["b_kj", "b_ji", "b_lin", "rb0_b1", "rb0_b2", "ra0_b1", "ra0_b2",
          "ra1_b1", "ra1_b2"]
    B = {n: din(n, [P, 1], F32) for n in bn}

    table = nc.dram_tensor("table", [etot, INT], F16).ap()
    outt = nc.dram_tensor("outt", [P, eperc], F32, kind="ExternalOutput").ap()

    Silu = mybir.ActivationFunctionType.Silu
    MUL, ADD, EQ = (mybir.AluOpType.mult, mybir.AluOpType.add,
                    mybir.AluOpType.is_equal)

    with tile.TileContext(nc) as tc, ExitStack() as ctx:
        cp = ctx.enter_context(tc.tile_pool(name="const", bufs=1))

        wsb = {}
        for n in wn:
            t = cp.tile(wshape.get(n, [HID, HID]), F32, tag=f"w_{n}")
            nc.sync.dma_start(out=t[:], in_=W[n][:, :])
            wsb[n] = t
        bsb = {}
        for n in bn:
            t = cp.tile([P, 1], F32, tag=f"b_{n}")
            nc.sync.dma_start(out=t[:], in_=B[n][:, :])
            bsb[n] = t
        iot = cp.tile([P, P], F32, tag="iota")
        nc.sync.dma_start(out=iot[:], in_=iota[:, :])
        idx_sb = cp.tile([P, nchunk], I32, tag="idxg")
        nc.sync.dma_start(out=idx_sb[:], in_=idxg[:, :])
        jig_sb = cp.tile([P, nchunk], F32, tag="jig")
        nc.sync.dma_start(out=jig_sb[:], in_=jig[:, :])

        # fp16 casts + host-folded weight products
        wkj16 = cp.tile([HID, HID], F16, tag="wkj16")
        nc.vector.tensor_copy(out=wkj16[:], in_=wsb["w_kj"][:])
        wdn16 = cp.tile([HID, INT], F16, tag="wdn16")
        nc.vector.tensor_copy(out=wdn16[:], in_=wsb["w_down"][:])
        wsb2 = cp.tile([64 + SR, INT], F16, tag="Wsb2")
        nc.sync.dma_start(out=wsb2[:], in_=Wsb2[:, :])
        wr3 = cp.tile([64 + NR, HID], F16, tag="Wr3")
        nc.sync.dma_start(out=wr3[:], in_=Wr3[:, :])

        # ---------------- Phase A: replicated gather-table build ------------
        with tc.tile_pool(name="pa_sb", bufs=3) as pa, \
             tc.tile_pool(name="pa_ps", bufs=2, space="PSUM") as pap, \
             tc.tile_pool(name="pa_io", bufs=2) as pio:
            xa = rt = tb = None
            for s in range(nsb_a):
                if s % 4 == 0:
                    xa = pio.tile([P, 2048], F16, tag="xa")
                    nc.sync.dma_start(out=xa[:], in_=xt16[:, s * 512:s * 512 + 2048])
                if s % 3 == 0:
                    rt = pio.tile([64 + NR, 512], F16, tag="rt")
                    nc.sync.dma_start(out=rt[:], in_=rbfp[:, (s // 3) * 512:(s // 3 + 1) * 512])
                xcols = xa[:, (s % 4) * 512:(s % 4) * 512 + 512]
                ps1 = pap.tile([P, 512], F32, tag="ps1", space="PSUM")
                nc.tensor.matmul(ps1[:], wkj16[:], xcols, start=True, stop=True)
                xkj = pa.tile([P, 512], F16, tag="xkj")
                nc.scalar.activation(xkj[:], ps1[:], Silu, bias=bsb["b_kj"][:])
                m = s % 3
                ps2 = pap.tile([P, 512], F32, tag="ps2", space="PSUM")
                nc.tensor.matmul(ps2[:], wr3[32 * m:32 * m + NR, :], rt[32 * m:32 * m + NR, :],
                                 start=True, stop=True)
                xkm = pa.tile([P, 512], F16, tag="xkm")
                nc.vector.tensor_tensor(out=xkm[:], in0=xkj[:], in1=ps2[:], op=MUL)
                if s % 2 == 0:
                    tb = pa.tile([P, 512], F16, tag="tb")
                for j in range(4):
                    ps3 = pap.tile([P, INT], F32, tag="ps3", space="PSUM")
                    nc.tensor.matmul(ps3[:], xkm[:, j * P:(j + 1) * P], wdn16[:],
                                     start=True, stop=True)
                    nc.scalar.activation(
                        tb[:, (s % 2) * 256 + j * INT:(s % 2) * 256 + (j + 1) * INT],
                        ps3[:], Silu)
                if s % 2 == 1:
                    dst = table[(s - 1) * 512:(s + 1) * 512, :].rearrange(
                        "(j p) f -> p j f", p=P)
                    src = tb[:].rearrange("p (j f) -> p j f", f=INT)
                    nc.sync.dma_start(out=dst, in_=src)

        # ---------------- Phase B + C: gather/scatter + dense tail ----------
        with tc.tile_pool(name="pb_sb", bufs=3) as pb, \
             tc.tile_pool(name="pb_big", bufs=2) as pbig, \
             tc.tile_pool(name="pb_ps", bufs=2, space="PSUM") as pbp, \
             tc.tile_pool(name="pc_sb", bufs=2) as pc, \
             tc.tile_pool(name="pc_ps", bufs=2, space="PSUM") as pcp:
            csb = 4 * K  # chunks per C-superblock
            for s in range(nsb_c):
                gt = pbig.tile([P, csb * INT], F16, tag="gt")
                nc.gpsimd.indirect_dma_start(
                    out=gt[:], out_offset=None, in_=table[:, :],
                    in_offset=bass.IndirectOffsetOnAxis(
                        ap=idx_sb[:, s * csb:(s + 1) * csb], axis=0))
                st = pbig.tile([64 + SR, (csb // 2) * P], F16, tag="st")
                nc.sync.dma_start(
                    out=st[:],
                    in_=sbft[:, s * (csb // 2) * P:(s + 1) * (csb // 2) * P])
                aggs = pc.tile([INT, 512], F32, tag="aggs")
                for j in range(4):
                    pagg = pbp.tile([INT, P], F32, tag="pagg", space="PSUM")
                    for k in range(K):
                        cc = j * K + k
                        ch = s * csb + cc
                        pse = pbp.tile([P, INT], F32, tag="pse", space="PSUM")
                        nc.tensor.matmul(
                            pse[:],
                            st[64 * (cc % 2):64 * (cc % 2) + SR,
                               (cc // 2) * P:(cc // 2 + 1) * P],
                            wsb2[64 * (cc % 2):64 * (cc % 2) + SR, :],
                            start=True, stop=True)
                        oh = pb.tile([P, P], F16, tag="oh")
                        nc.vector.tensor_scalar(
                            out=oh[:], in0=iot[:], scalar1=jig_sb[:, ch:ch + 1],
                            scalar2=None, op0=EQ)
                        msg = pb.tile([P, INT], F16, tag="msg")
                        nc.vector.tensor_tensor(
                            out=msg[:], in0=gt[:, cc * INT:(cc + 1) * INT],
                            in1=pse[:], op=MUL)
                        nc.tensor.matmul(pagg[:], msg[:], oh[:],
                                         start=(k == 0), stop=(k == K - 1))
                    nc.vector.tensor_copy(out=aggs[:, j * P:(j + 1) * P],
                                          in_=pagg[:])
                # dense tail on 512 edges (feature-transposed, fp32)
                pup = pcp.tile([P, 512], F32, tag="psC", space="PSUM")
                nc.tensor.matmul(pup[:], wsb["w_up"][:], aggs[:],
                                 start=True, stop=True)
                xup = pc.tile([P, 512], F32, tag="xup")
                nc.scalar.activation(xup[:], pup[:], Silu)
                xl = pc.tile([P, 512], F32, tag="xl")
                nc.sync.dma_start(out=xl[:], in_=xt32[:, s * 512:(s + 1) * 512])
                pji = pcp.tile([P, 512], F32, tag="psC", space="PSUM")
                nc.tensor.matmul(pji[:], wsb["w_ji"][:], xl[:],
                                 start=True, stop=True)
                hji = pc.tile([P, 512], F32, tag="hji")
                nc.scalar.activation(hji[:], pji[:], Silu, bias=bsb["b_ji"][:])
                h = pc.tile([P, 512], F32, tag="h0")
                nc.vector.tensor_tensor(out=h[:], in0=hji[:], in1=xup[:], op=ADD)

                def res(hin, w1, b1, w2, b2, tg):
                    p1 = pcp.tile([P, 512], F32, tag="psC", space="PSUM")
                    nc.tensor.matmul(p1[:], wsb[w1][:], hin[:], start=True, stop=True)
                    t1 = pc.tile([P, 512], F32, tag=f"t1{tg}")
                    nc.scalar.activation(t1[:], p1[:], Silu, bias=bsb[b1][:])
                    p2 = pcp.tile([P, 512], F32, tag="psC", space="PSUM")
                    nc.tensor.matmul(p2[:], wsb[w2][:], t1[:], start=True, stop=True)
                    t2 = pc.tile([P, 512], F32, tag=f"t2{tg}")
                    nc.scalar.activation(t2[:], p2[:], Silu, bias=bsb[b2][:])
                    ho = pc.tile([P, 512], F32, tag=f"h{tg}")
                    nc.vector.tensor_tensor(out=ho[:], in0=hin[:], in1=t2[:], op=ADD)
                    return ho

                h = res(h, "rb0_w1", "rb0_b1", "rb0_w2", "rb0_b2", "r0")
                pl = pcp.tile([P, 512], F32, tag="psC", space="PSUM")
                nc.tensor.matmul(pl[:], wsb["w_lin"][:], h[:], start=True, stop=True)
                hl = pc.tile([P, 512], F32, tag="hl")
                nc.scalar.activation(hl[:], pl[:], Silu, bias=bsb["b_lin"][:])
                h = pc.tile([P, 512], F32, tag="h1")
                nc.vector.tensor_tensor(out=h[:], in0=hl[:], in1=xl[:], op=ADD)
                h = res(h, "ra0_w1", "ra0_b1", "ra0_w2", "ra0_b2", "a0")
                h = res(h, "ra1_w1", "ra1_b1", "ra1_w2", "ra1_b2", "a1")
                nc.sync.dma_start(out=outt[:, s * 512:(s + 1) * 512], in_=h[:])
    return outt


# ----------------------------------------------------------------------------
def _run(inputs, n_cores=8, trace=False):
    x = np.asarray(inputs["x"], np.float32)
    rbf = np.asarray(inputs["rbf"], np.float32)
    sbf = np.asarray(inputs["sbf"], np.float32)
    idx_kj = np.asarray(inputs["idx_kj"])
    idx_ji = np.asarray(inputs["idx_ji"])

    d, shared, per_core, new_global = _preprocess(
        x, rbf, sbf, idx_kj, idx_ji, n_cores)

    nc = bacc.Bacc("TRN2", target_bir_lowering=False, debug=False,
                   enable_asserts=False, num_devices=n_cores)
    _build(nc, d)
    nc.compile()

    f32 = np.float32
    base = dict(shared)
    base["iota"] = shared["iota"]
    base["w_kj"] = np.ascontiguousarray(inputs["w_kj"], f32)
    base["w_down"] = np.ascontiguousarray(inputs["w_down"], f32)
    wsb_f = (np.asarray(inputs["w_sbf1"], f32) @
             np.asarray(inputs["w_sbf2"], f32)).astype(np.float16)
    Wsb2 = np.zeros((64 + SR, INT), dtype=np.float16)
    Wsb2[0:SR] = wsb_f
    Wsb2[64:64 + SR] = wsb_f
    base["Wsb2"] = Wsb2
    wr_f = (np.asarray(inputs["w_rbf1"], f32) @
            np.asarray(inputs["w_rbf2"], f32)).astype(np.float16)
    Wr3 = np.zeros((64 + NR, HID), dtype=np.float16)
    for g in range(3):
        Wr3[32 * g:32 * g + NR] = wr_f
    base["Wr3"] = Wr3
    base["w_ji"] = np.ascontiguousarray(inputs["w_ji"], f32)
    base["w_up"] = np.ascontiguousarray(inputs["w_up"], f32)
    base["rb0_w1"] = np.ascontiguousarray(inputs["rb0_w1"], f32)
    base["rb0_w2"] = np.ascontiguousarray(inputs["rb0_w2"], f32)
    base["w_lin"] = np.ascontiguousarray(inputs["w_lin"], f32)
    base["ra0_w1"] = np.ascontiguousarray(inputs["ra0_w1"], f32)
    base["ra0_w2"] = np.ascontiguousarray(inputs["ra0_w2"], f32)
    base["ra1_w1"] = np.ascontiguousarray(inputs["ra1_w1"], f32)
    base["ra1_w2"] = np.ascontiguousarray(inputs["ra1_w2"], f32)
    for bnm in ["b_kj", "b_ji", "b_lin", "rb0_b1", "rb0_b2", "ra0_b1",
                "ra0_b2", "ra1_b1", "ra1_b2"]:
        base[bnm] = np.ascontiguousarray(
            np.asarray(inputs[bnm], f32).reshape(P, 1))

    in_maps = []
    for c in range(n_cores):
        m = dict(base)
        m.update(per_core[c])
        in_maps.append(m)

    res = run_bass_kernel_spmd(nc, in_maps, core_ids=list(range(n_cores)),
                               trace=trace)
    h_full = np.concatenate([res.results[c]["outt"].T for c in range(n_cores)],
                            axis=0)
    out = h_full[new_global].astype(np.float32)
    return out, res


def kernel(**inputs):
    out, _ = _run(inputs, n_cores=8, trace=False)
    return out



# revision 14
# speedup vs baseline: 2.6539x; 2.6539x over previous
"""Trainium2 Bass kernel for an InteractionPPBlock-style GNN message-passing layer.

Strategy (8 NeuronCores):
  * Edges partitioned 25000/core by idx_ji ownership; triplets land on the core
    that owns their scatter destination, so segment_sum is local (one-hot
    matmul into PSUM).
  * Host-side graph partitioning: each core's edges are dealt into 196 bins of
    128 slots with degree-balanced bin sums (snake deal + swap fixups) so every
    bin holds exactly K*128 triplets after padding -> uniform SPMD program.
  * Phase A (sharded): each core computes the gather table
    x_kjd = swish((swish(x@w_kj+b_kj)*rbf_e) @ w_down) for ITS OWN 25088 edges
    only (fp8 output), then an AllGather replicates the full fp8 table to every
    core's DRAM.  Gather is an indirect DMA of 64B rows.
  * Dense per-edge tail (x_ji, w_up, residual blocks) runs in fp16 on
    feature-transposed 1024-wide tiles; weights pre-cast to fp16 on host.
"""

import math
from contextlib import ExitStack

import numpy as np
import ml_dtypes

import concourse.bass as bass
import concourse.mybir as mybir
import concourse.tile as tile
from concourse import bacc
from concourse.bass_utils import run_bass_kernel_spmd

F32 = mybir.dt.float32
F16 = mybir.dt.float16
F8 = mybir.dt.float8e4
I32 = mybir.dt.int32

NP_F8 = ml_dtypes.float8_e4m3fn

HID, INT, BAS, NR, NS = 128, 64, 8, 6, 7
SR = NS * NR  # 42
P = 128

TABLE_F8 = True  # gather table dtype: fp8e4m3 (else fp16)
ACT_FN = mybir.ActivationFunctionType.Silu  # swapped for Sigmoid in sim tests


# ----------------------------------------------------------------------------
# Host-side graph partitioning (free: runs in numpy, not on device)
# ----------------------------------------------------------------------------
def _balance_bins(deg, nblk, cap):
    """Assign len(deg) edges to nblk bins of exactly <=P edges, minimizing the
    max bin degree-sum; returns per-edge bin id."""
    n = len(deg)
    order = np.argsort(-deg, kind="stable")
    r = np.arange(n)
    q, pos = r // nblk, r % nblk
    bins = np.where(q % 2 == 0, pos, nblk - 1 - pos)
    asn = np.empty(n, dtype=np.int64)
    asn[order] = bins
    binsum = np.bincount(asn, weights=deg.astype(np.float64), minlength=nblk)
    for _ in range(2000):
        bmax = int(binsum.argmax())
        if binsum[bmax] <= cap:
            break
        bmin = int(binsum.argmin())
        need = binsum[bmax] - cap
        room = cap - binsum[bmin]
        ii = np.nonzero(asn == bmax)[0]
        jj = np.nonzero(asn == bmin)[0]
        delta = deg[ii][:, None].astype(np.int64) - deg[jj][None, :]
        ok = (delta > 0) & (delta <= room) & (delta >= min(need, room))
        if ok.any():
            a, b = np.argwhere(ok)[0]
        else:
            d2 = np.where(delta <= room, delta, -1)
            a, b = np.unravel_index(np.argmax(d2), d2.shape)
            if d2[a, b] <= 0:
                break
        i, j = ii[a], jj[b]
        asn[i], asn[j] = bmin, bmax
        d = deg[i] - deg[j]
        binsum[bmax] -= d
        binsum[bmin] += d
    return asn, int(binsum.max())


def _preprocess(x, rbf, sbf, idx_kj, idx_ji, n_cores):
    E = x.shape[0]
    T = sbf.shape[0]
    eper = E // n_cores                      # original edges per core
    nblk = math.ceil(eper / P)
    if nblk % 4:
        nblk += 4 - nblk % 4
    eperc = nblk * P                         # padded edges per core
    etot = n_cores * eperc

    idx_kj = idx_kj.astype(np.int64)
    idx_ji = idx_ji.astype(np.int64)
    owner_t = idx_ji // eper                 # triplet -> core
    deg = np.bincount(idx_ji, minlength=E)

    # degree-balanced binning per core (target K*P bin capacity)
    newlocal = np.empty(E, dtype=np.int64)
    binsum_max = 0
    for c in range(n_cores):
        lo, hi = c * eper, (c + 1) * eper
        asn, mx = _balance_bins(deg[lo:hi], nblk, 10 * P)
        binsum_max = max(binsum_max, mx)
        # slot = rank within bin (bins have <=P members by construction)
        o = np.argsort(asn, kind="stable")
        cnt = np.bincount(asn, minlength=nblk)
        starts = np.zeros(nblk, dtype=np.int64)
        starts[1:] = np.cumsum(cnt)[:-1]
        rank = np.empty(eper, dtype=np.int64)
        rank[o] = np.arange(eper) - np.repeat(starts, cnt)
        newlocal[lo:hi] = asn * P + rank

    K = max(1, math.ceil(binsum_max / P))
    cap = K * P
    nchunk = nblk * K
    tpad = nblk * cap
    new_global = (np.arange(E) // eper) * eperc + newlocal

    # table row layout: shard-major [core][partition p][block j][feat]
    nblkA = eperc // P
    eg = np.arange(etot)
    cg, lg = eg // eperc, eg % eperc
    table_row = cg * eperc + (lg % P) * nblkA + lg // P   # table row of edge e

    per_core = []
    for c in range(n_cores):
        tri = np.nonzero(owner_t == c)[0]
        nlji = newlocal[idx_ji[tri]]
        b_of, s_of = nlji // P, nlji % P
        o2 = np.argsort(b_of * P + s_of, kind="stable")
        tri, b_of, s_of = tri[o2], b_of[o2], s_of[o2]
        cnt = np.bincount(b_of, minlength=nblk)
        starts = np.zeros(nblk, dtype=np.int64)
        starts[1:] = np.cumsum(cnt)[:-1]
        rank = np.arange(len(tri)) - np.repeat(starts, cnt)
        pos = b_of * cap + rank

        kj_new = np.zeros(tpad, dtype=np.int32)
        kj_new[pos] = table_row[new_global[idx_kj[tri]]].astype(np.int32)
        ji_sh = np.zeros(tpad, dtype=np.float16)
        ji_sh[pos] = s_of.astype(np.float16)
        sbf_pad = np.zeros((tpad, SR), dtype=np.float16)
        sbf_pad[pos] = sbf[tri].astype(np.float16)

        idx_grid = np.ascontiguousarray(kj_new.reshape(nchunk, P).T)
        ji_grid = np.ascontiguousarray(ji_sh.reshape(nchunk, P).T)
        # sbf^T single-stack [42, nchunk*P] (keeps all PE operands at base
        # partition 0 -- base-64 stationary + sub-bank PSUM dst faults).
        sbf_t = np.ascontiguousarray(
            sbf_pad.reshape(nchunk * P, SR).T)
        per_core.append(dict(idxg=idx_grid, jig=ji_grid, sbft=sbf_t))

    # globally renumbered x / rbf
    x_g = np.zeros((etot, HID), dtype=np.float32)
    x_g[new_global] = x
    rbf_g = np.zeros((etot, NR), dtype=np.float32)
    rbf_g[new_global] = rbf

    nsb_a = eperc // 512                     # phase-A chunks per core (49)
    ngrp_r = math.ceil(nsb_a / 3)
    for c in range(n_cores):
        sl = slice(c * eperc, (c + 1) * eperc)
        per_core[c]["xt16c"] = np.ascontiguousarray(
            x_g[sl].T.astype(np.float16))
        # rbf^T 3-stacked at partition offsets {0, 32, 64}
        rbf_c = np.zeros((ngrp_r * 3 * 512, NR), dtype=np.float32)
        rbf_c[:eperc] = rbf_g[sl]
        Rt = (rbf_c.reshape(ngrp_r, 3, 512, NR).transpose(1, 3, 0, 2)
              .reshape(3, NR, ngrp_r * 512).astype(np.float16))
        rbfp = np.zeros((64 + NR, ngrp_r * 512), dtype=np.float16)
        for g in range(3):
            rbfp[32 * g:32 * g + NR] = Rt[g]
        per_core[c]["rbfpc"] = rbfp

    dims = dict(n_cores=n_cores, E=E, T=T, eper=eper, nblk=nblk, eperc=eperc,
                etot=etot, K=K, cap=cap, nchunk=nchunk, nsb_a=nsb_a,
                ngrp_r=ngrp_r, nblkA=nblkA)
    shared = dict(
        iota_w=np.tile(np.arange(P, dtype=np.float16), (P, 4)))
    return dims, shared, per_core, new_global


# ----------------------------------------------------------------------------
# Device program
# ----------------------------------------------------------------------------
def _build(nc, d):
    nblk, K, nchunk = d["nblk"], d["K"], d["nchunk"]
    eperc, nsb_a, ngrp_r = d["eperc"], d["nsb_a"], d["ngrp_r"]
    etot, nblkA = d["etot"], d["nblkA"]
    n_cores = d["n_cores"]
    nsb_c = nblk // 4
    TDT = F8 if TABLE_F8 else F16

    def din(name, shape, dt):
        return nc.dram_tensor(name, shape, dt, kind="ExternalInput").ap()

    xt16c = din("xt16c", [P, eperc], F16)
    rbfpc = din("rbfpc", [64 + NR, ngrp_r * 512], F16)
    iota_w = din("iota_w", [P, 4 * P], F16)
    sbft = din("sbft", [SR, nchunk * P], F16)
    idxg = din("idxg", [P, nchunk], I32)
    jig = din("jig", [P, nchunk], F16)

    # fp16 weights (host-cast); stacked rbf/sbf folded weights
    w16n = ["w_kj16", "w_down16", "w_ji16", "w_up16", "rb0_w116", "rb0_w216",
            "w_lin16", "ra0_w116", "ra0_w216", "ra1_w116", "ra1_w216"]
    wshape = dict(w_down16=[HID, INT], w_up16=[INT, HID])
    Wsb2 = din("Wsb2", [SR, INT], F16)
    Wr3 = din("Wr3", [64 + NR, HID], F16)
    W = {n: din(n, wshape.get(n, [HID, HID]), F16) for n in w16n}
    bn = ["b_kj", "b_ji", "b_lin", "rb0_b1", "rb0_b2", "ra0_b1", "ra0_b2",
          "ra1_b1", "ra1_b2"]
    B = {n: din(n, [P, 1], F32) for n in bn}

    shard = nc.dram_tensor("shard", [P, nblkA * INT], TDT).ap()
    table = nc.dram_tensor("table", [etot, INT], TDT).ap()
    outt = nc.dram_tensor("outt", [P, eperc], F16, kind="ExternalOutput").ap()

    Silu = ACT_FN
    MUL, ADD, EQ = (mybir.AluOpType.mult, mybir.AluOpType.add,
                    mybir.AluOpType.is_equal)

    with tile.TileContext(nc) as tc, ExitStack() as ctx:
        cp = ctx.enter_context(tc.tile_pool(name="const", bufs=1))

        wsb = {}
        for n in w16n:
            t = cp.tile(wshape.get(n, [HID, HID]), F16, tag=n)
            nc.sync.dma_start(out=t[:], in_=W[n][:, :])
            wsb[n] = t
        bsb = {}
        for n in bn:
            t = cp.tile([P, 1], F32, tag=f"b_{n}")
            nc.sync.dma_start(out=t[:], in_=B[n][:, :])
            bsb[n] = t
        iot = cp.tile([P, 4 * P], F16, tag="iota")
        nc.sync.dma_start(out=iot[:], in_=iota_w[:, :])
        idx_sb = cp.tile([P, nchunk], I32, tag="idxg")
        nc.sync.dma_start(out=idx_sb[:], in_=idxg[:, :])
        jig_sb = cp.tile([P, nchunk], F16, tag="jig")
        nc.sync.dma_start(out=jig_sb[:], in_=jig[:, :])
        wsb2 = cp.tile([SR, INT], F16, tag="Wsb2")
        nc.sync.dma_start(out=wsb2[:], in_=Wsb2[:, :])
        wr3 = cp.tile([64 + NR, HID], F16, tag="Wr3")
        nc.sync.dma_start(out=wr3[:], in_=Wr3[:, :])
        # resident x (feature-major fp16), used by phases A and C
        xsb = cp.tile([P, eperc], F16, tag="xsb")
        half = eperc // 2
        nc.sync.dma_start(out=xsb[:, :half], in_=xt16c[:, :half])
        nc.sync.dma_start(out=xsb[:, half:], in_=xt16c[:, half:])

        # ---------------- Phase A: sharded gather-table build ---------------
        tbfull = cp.tile([P, nblkA * INT], TDT, tag="tbfull")
        with tc.tile_pool(name="pa_sb", bufs=3) as pa, \
             tc.tile_pool(name="pa_ps", bufs=2, space="PSUM") as pap, \
             tc.tile_pool(name="pa_io", bufs=2) as pio:
            rt = None
            for s in range(nsb_a):
                if s % 3 == 0:
                    rt = pio.tile([64 + NR, 512], F16, tag="rt")
                    nc.sync.dma_start(
                        out=rt[:],
                        in_=rbfpc[:, (s // 3) * 512:(s // 3 + 1) * 512])
                ps1 = pap.tile([P, 512], F32, tag="ps1", space="PSUM")
                nc.tensor.matmul(ps1[:], wsb["w_kj16"][:],
                                 xsb[:, s * 512:(s + 1) * 512],
                                 start=True, stop=True)
                xkj = pa.tile([P, 512], F16, tag="xkj")
                nc.scalar.activation(xkj[:], ps1[:], Silu, bias=bsb["b_kj"][:])
                m = s % 3
                ps2 = pap.tile([P, 512], F32, tag="ps2", space="PSUM")
                nc.tensor.matmul(ps2[:], wr3[32 * m:32 * m + NR, :],
                                 rt[32 * m:32 * m + NR, :],
                                 start=True, stop=True)
                xkm = pa.tile([P, 512], F16, tag="xkm")
                nc.vector.tensor_tensor(out=xkm[:], in0=xkj[:], in1=ps2[:],
                                        op=MUL)
                pd = pap.tile([P, 4 * INT], F32, tag="pd", space="PSUM")
                for j in range(4):
                    nc.tensor.matmul(pd[:, j * INT:(j + 1) * INT],
                                     xkm[:, j * P:(j + 1) * P],
                                     wsb["w_down16"][:],
                                     start=True, stop=True)
                nc.scalar.activation(
                    tbfull[:, s * 4 * INT:(s + 1) * 4 * INT], pd[:], Silu)
            nc.sync.dma_start(out=shard[:, :], in_=tbfull[:])

        # ---------------- AllGather: replicate table shards ------------------
        nc.gpsimd.collective_compute(
            "AllGather", mybir.AluOpType.bypass,
            replica_groups=[list(range(n_cores))],
            ins=[shard[:, :].opt()], outs=[table[:, :].opt()])
        # CC cores run collectives in order; a barrier AllReduce whose
        # output is read back makes the table delivery observable.
        bar_in = nc.dram_tensor("bar_in", [P, 4], F32).ap()
        bar_out = nc.dram_tensor("bar_out", [P, 4], F32).ap()
        bar_sb = cp.tile([P, 4], F32, tag="bar_sb")
        nc.gpsimd.memset(bar_sb[:], 0.0)
        nc.gpsimd.dma_start(out=bar_in[:, :], in_=bar_sb[:])
        nc.gpsimd.collective_compute(
            "AllReduce", mybir.AluOpType.add,
            replica_groups=[list(range(n_cores))],
            ins=[bar_in[:, :].opt()], outs=[bar_out[:, :].opt()])
        bar_rd = cp.tile([P, 4], F32, tag="bar_rd")
        nc.gpsimd.dma_start(out=bar_rd[:], in_=bar_out[:, :])

        # ---------------- Phase B + C: gather/scatter + dense tail ----------
        with tc.tile_pool(name="pb_sb", bufs=3) as pb, \
             tc.tile_pool(name="pb_big", bufs=2) as pbig, \
             tc.tile_pool(name="pb_ps", bufs=2, space="PSUM") as pbp, \
             tc.tile_pool(name="pc_sb", bufs=2) as pc, \
             tc.tile_pool(name="pc_ps", bufs=2, space="PSUM") as pcp:
            csb = 4 * K  # chunks per superblock
            ngr = math.ceil(K / 4)
            aggs = None

            def mm(ps, w, rhs, W):
                # matmul moving-dim/PSUM-bank limit: emit in 512-col pieces
                for o in range(0, W, 512):
                    nc.tensor.matmul(ps[:, o:o + 512], w,
                                     rhs[:, o:o + 512], start=True, stop=True)

            def tail(cols, W):
                """dense per-edge tail over W edges (feature-major fp16)."""
                xl = xsb[:, cols]
                pup = pcp.tile([P, 1024], F32, tag="psC", space="PSUM")
                mm(pup, wsb["w_up16"][:], aggs, W)
                xup = pc.tile([P, 1024], F16, tag="xup")
                nc.scalar.activation(xup[:, :W], pup[:, :W], Silu)
                pji = pcp.tile([P, 1024], F32, tag="psC", space="PSUM")
                mm(pji, wsb["w_ji16"][:], xl, W)
                hji = pc.tile([P, 1024], F16, tag="hji")
                nc.scalar.activation(hji[:, :W], pji[:, :W], Silu,
                                     bias=bsb["b_ji"][:])
                h = pc.tile([P, 1024], F16, tag="h0")
                nc.vector.tensor_tensor(out=h[:, :W], in0=hji[:, :W],
                                        in1=xup[:, :W], op=ADD)

                def res(hin, w1, b1, w2, b2, tg):
                    p1 = pcp.tile([P, 1024], F32, tag="psC", space="PSUM")
                    mm(p1, wsb[w1][:], hin, W)
                    t1 = pc.tile([P, 1024], F16, tag=f"t1{tg}")
                    nc.scalar.activation(t1[:, :W], p1[:, :W], Silu,
                                         bias=bsb[b1][:])
                    p2 = pcp.tile([P, 1024], F32, tag="psC", space="PSUM")
                    mm(p2, wsb[w2][:], t1, W)
                    t2 = pc.tile([P, 1024], F16, tag=f"t2{tg}")
                    nc.scalar.activation(t2[:, :W], p2[:, :W], Silu,
                                         bias=bsb[b2][:])
                    ho = pc.tile([P, 1024], F16, tag=f"h{tg}")
                    nc.vector.tensor_tensor(out=ho[:, :W], in0=hin[:, :W],
                                            in1=t2[:, :W], op=ADD)
                    return ho

                h = res(h, "rb0_w116", "rb0_b1", "rb0_w216", "rb0_b2", "r0")
                pl = pcp.tile([P, 1024], F32, tag="psC", space="PSUM")
                mm(pl, wsb["w_lin16"][:], h, W)
                hl = pc.tile([P, 1024], F16, tag="hl")
                nc.scalar.activation(hl[:, :W], pl[:, :W], Silu,
                                     bias=bsb["b_lin"][:])
                h = pc.tile([P, 1024], F16, tag="h1")
                nc.vector.tensor_tensor(out=h[:, :W], in0=hl[:, :W], in1=xl,
                                        op=ADD)
                h = res(h, "ra0_w116", "ra0_b1", "ra0_w216", "ra0_b2", "a0")
                h = res(h, "ra1_w116", "ra1_b1", "ra1_w216", "ra1_b2", "a1")
                nc.sync.dma_start(out=outt[:, cols], in_=h[:, :W])

            for s in range(nsb_c):
                gt = pbig.tile([P, csb * INT], TDT, tag="gt")
                nc.gpsimd.indirect_dma_start(
                    out=gt[:], out_offset=None, in_=table[:, :],
                    in_offset=bass.IndirectOffsetOnAxis(
                        ap=idx_sb[:, s * csb:(s + 1) * csb], axis=0))
                st = pbig.tile([SR, csb * P], F16, tag="st")
                nc.sync.dma_start(out=st[:],
                                  in_=sbft[:, s * csb * P:(s + 1) * csb * P])
                if s % 2 == 0:
                    aggs = pc.tile([INT, 1024], F16, tag="aggs")
                for j in range(4):
                    pagg = pbp.tile([INT, P], F32, tag="pagg", space="PSUM")
                    for kg in range(ngr):
                        g = min(4, K - 4 * kg)
                        k0 = 4 * kg
                        psE = pbp.tile([P, 4 * INT], F32, tag="psE",
                                       space="PSUM")
                        for q in range(g):
                            cc = j * K + k0 + q
                            nc.tensor.matmul(
                                psE[:, q * INT:(q + 1) * INT],
                                st[:, cc * P:(cc + 1) * P],
                                wsb2[:],
                                start=True, stop=True)
                        ch0 = s * csb + j * K + k0
                        oh4 = pb.tile([P, 4 * P], F16, tag="oh")
                        nc.vector.tensor_tensor(
                            out=oh4[:, :g * P].rearrange(
                                "p (c w) -> p c w", w=P),
                            in0=iot[:, :g * P].rearrange(
                                "p (c w) -> p c w", w=P),
                            in1=jig_sb[:, ch0:ch0 + g].broadcast_to(
                                [P, g, P]),
                            op=EQ)
                        msg4 = pb.tile([P, 4 * INT], F16, tag="msg")
                        gt0 = (j * K + k0) * INT
                        nc.vector.tensor_tensor(
                            out=msg4[:, :g * INT],
                            in0=gt[:, gt0:gt0 + g * INT],
                            in1=psE[:, :g * INT], op=MUL)
                        for q in range(g):
                            nc.tensor.matmul(
                                pagg[:], msg4[:, q * INT:(q + 1) * INT],
                                oh4[:, q * P:(q + 1) * P],
                                start=(kg == 0 and q == 0),
                                stop=(k0 + q == K - 1))
                    nc.vector.tensor_copy(
                        out=aggs[:, (s % 2) * 512 + j * P:
                                 (s % 2) * 512 + (j + 1) * P],
                        in_=pagg[:])
                if s % 2 == 1:
                    tail(slice((s - 1) * 512, (s + 1) * 512), 1024)
                elif s == nsb_c - 1:
                    tail(slice(s * 512, (s + 1) * 512), 512)
    return outt


# ----------------------------------------------------------------------------
def _run(inputs, n_cores=8, trace=False):
    x = np.asarray(inputs["x"], np.float32)
    rbf = np.asarray(inputs["rbf"], np.float32)
    sbf = np.asarray(inputs["sbf"], np.float32)
    idx_kj = np.asarray(inputs["idx_kj"])
    idx_ji = np.asarray(inputs["idx_ji"])

    d, shared, per_core, new_global = _preprocess(
        x, rbf, sbf, idx_kj, idx_ji, n_cores)

    nc = bacc.Bacc("TRN2", target_bir_lowering=False, debug=False,
                   enable_asserts=False, num_devices=n_cores)
    _build(nc, d)
    nc.compile()

    f32, f16 = np.float32, np.float16
    base = dict(shared)
    wmap = dict(w_kj16="w_kj", w_down16="w_down", w_ji16="w_ji",
                w_up16="w_up", rb0_w116="rb0_w1", rb0_w216="rb0_w2",
                w_lin16="w_lin", ra0_w116="ra0_w1", ra0_w216="ra0_w2",
                ra1_w116="ra1_w1", ra1_w216="ra1_w2")
    for k16, k in wmap.items():
        base[k16] = np.ascontiguousarray(np.asarray(inputs[k], f32), f32
                                         ).astype(f16)
    wsb_f = (np.asarray(inputs["w_sbf1"], f32) @
             np.asarray(inputs["w_sbf2"], f32)).astype(f16)
    base["Wsb2"] = np.ascontiguousarray(wsb_f)
    wr_f = (np.asarray(inputs["w_rbf1"], f32) @
            np.asarray(inputs["w_rbf2"], f32)).astype(f16)
    Wr3 = np.zeros((64 + NR, HID), dtype=f16)
    for g in range(3):
        Wr3[32 * g:32 * g + NR] = wr_f
    base["Wr3"] = Wr3
    for bnm in ["b_kj", "b_ji", "b_lin", "rb0_b1", "rb0_b2", "ra0_b1",
                "ra0_b2", "ra1_b1", "ra1_b2"]:
        base[bnm] = np.ascontiguousarray(
            np.asarray(inputs[bnm], f32).reshape(P, 1))

    in_maps = []
    for c in range(n_cores):
        m = dict(base)
        m.update(per_core[c])
        in_maps.append(m)

    res = run_bass_kernel_spmd(nc, in_maps, core_ids=list(range(n_cores)),
                               trace=trace)
    h_full = np.concatenate([res.results[c]["outt"].T for c in range(n_cores)],
                            axis=0)
    out = h_full[new_global].astype(np.float32)
    return out, res


def kernel(**inputs):
    out, _ = _run(inputs, n_cores=8, trace=False)
    return out


# revision 15
# speedup vs baseline: 2.7312x; 1.0291x over previous
"""Trainium2 Bass kernel for an InteractionPPBlock-style GNN message-passing layer.

Strategy (8 NeuronCores):
  * Edges partitioned 25000/core by idx_ji ownership; triplets land on the core
    that owns their scatter destination, so segment_sum is local (one-hot
    matmul into PSUM).
  * Host-side graph partitioning: each core's edges are dealt into 196 bins of
    128 slots with degree-balanced bin sums (snake deal + swap fixups) so every
    bin holds exactly K*128 triplets after padding -> uniform SPMD program.
  * Phase A (sharded): each core computes the gather table
    x_kjd = swish((swish(x@w_kj+b_kj)*rbf_e) @ w_down) for ITS OWN 25088 edges
    only (fp8 output), then an AllGather replicates the full fp8 table to every
    core's DRAM.  Gather is an indirect DMA of 64B rows.
  * Dense per-edge tail (x_ji, w_up, residual blocks) runs in fp16 on
    feature-transposed 1024-wide tiles; weights pre-cast to fp16 on host.
"""

import math
from contextlib import ExitStack

import numpy as np
import ml_dtypes

import concourse.bass as bass
import concourse.mybir as mybir
import concourse.tile as tile
from concourse import bacc
from concourse.bass_utils import run_bass_kernel_spmd

F32 = mybir.dt.float32
F16 = mybir.dt.float16
F8 = mybir.dt.float8e4
I32 = mybir.dt.int32

NP_F8 = ml_dtypes.float8_e4m3fn

HID, INT, BAS, NR, NS = 128, 64, 8, 6, 7
SR = NS * NR  # 42
P = 128

TABLE_F8 = True  # gather table dtype: fp8e4m3 (else fp16)
ACT_FN = mybir.ActivationFunctionType.Silu  # swapped for Sigmoid in sim tests


# ----------------------------------------------------------------------------
# Host-side graph partitioning (free: runs in numpy, not on device)
# ----------------------------------------------------------------------------
def _balance_bins(deg, nblk, cap):
    """Assign len(deg) edges to nblk bins of exactly <=P edges, minimizing the
    max bin degree-sum; returns per-edge bin id."""
    n = len(deg)
    order = np.argsort(-deg, kind="stable")
    r = np.arange(n)
    q, pos = r // nblk, r % nblk
    bins = np.where(q % 2 == 0, pos, nblk - 1 - pos)
    asn = np.empty(n, dtype=np.int64)
    asn[order] = bins
    binsum = np.bincount(asn, weights=deg.astype(np.float64), minlength=nblk)
    for _ in range(2000):
        bmax = int(binsum.argmax())
        if binsum[bmax] <= cap:
            break
        bmin = int(binsum.argmin())
        need = binsum[bmax] - cap
        room = cap - binsum[bmin]
        ii = np.nonzero(asn == bmax)[0]
        jj = np.nonzero(asn == bmin)[0]
        delta = deg[ii][:, None].astype(np.int64) - deg[jj][None, :]
        ok = (delta > 0) & (delta <= room) & (delta >= min(need, room))
        if ok.any():
            a, b = np.argwhere(ok)[0]
        else:
            d2 = np.where(delta <= room, delta, -1)
            a, b = np.unravel_index(np.argmax(d2), d2.shape)
            if d2[a, b] <= 0:
                break
        i, j = ii[a], jj[b]
        asn[i], asn[j] = bmin, bmax
        d = deg[i] - deg[j]
        binsum[bmax] -= d
        binsum[bmin] += d
    return asn, int(binsum.max())


def _preprocess(x, rbf, sbf, idx_kj, idx_ji, n_cores):
    E = x.shape[0]
    T = sbf.shape[0]
    eper = E // n_cores                      # original edges per core
    nblk = math.ceil(eper / P)
    if nblk % 4:
        nblk += 4 - nblk % 4
    eperc = nblk * P                         # padded edges per core
    etot = n_cores * eperc

    idx_kj = idx_kj.astype(np.int64)
    idx_ji = idx_ji.astype(np.int64)
    owner_t = idx_ji // eper                 # triplet -> core
    deg = np.bincount(idx_ji, minlength=E)

    # degree-balanced binning per core (target K*P bin capacity)
    newlocal = np.empty(E, dtype=np.int64)
    binsum_max = 0
    for c in range(n_cores):
        lo, hi = c * eper, (c + 1) * eper
        asn, mx = _balance_bins(deg[lo:hi], nblk, 10 * P)
        binsum_max = max(binsum_max, mx)
        # slot = rank within bin (bins have <=P members by construction)
        o = np.argsort(asn, kind="stable")
        cnt = np.bincount(asn, minlength=nblk)
        starts = np.zeros(nblk, dtype=np.int64)
        starts[1:] = np.cumsum(cnt)[:-1]
        rank = np.empty(eper, dtype=np.int64)
        rank[o] = np.arange(eper) - np.repeat(starts, cnt)
        newlocal[lo:hi] = asn * P + rank

    K = max(1, math.ceil(binsum_max / P))
    cap = K * P
    nchunk = nblk * K
    tpad = nblk * cap
    new_global = (np.arange(E) // eper) * eperc + newlocal

    # table row layout: shard-major [core][partition p][block j][feat]
    nblkA = eperc // P
    eg = np.arange(etot)
    cg, lg = eg // eperc, eg % eperc
    table_row = cg * eperc + (lg % P) * nblkA + lg // P   # table row of edge e

    per_core = []
    for c in range(n_cores):
        tri = np.nonzero(owner_t == c)[0]
        nlji = newlocal[idx_ji[tri]]
        b_of, s_of = nlji // P, nlji % P
        o2 = np.argsort(b_of * P + s_of, kind="stable")
        tri, b_of, s_of = tri[o2], b_of[o2], s_of[o2]
        cnt = np.bincount(b_of, minlength=nblk)
        starts = np.zeros(nblk, dtype=np.int64)
        starts[1:] = np.cumsum(cnt)[:-1]
        rank = np.arange(len(tri)) - np.repeat(starts, cnt)
        pos = b_of * cap + rank

        kj_new = np.zeros(tpad, dtype=np.int32)
        kj_new[pos] = table_row[new_global[idx_kj[tri]]].astype(np.int32)
        ji_sh = np.zeros(tpad, dtype=np.float16)
        ji_sh[pos] = s_of.astype(np.float16)
        sbf_pad = np.zeros((tpad, SR), dtype=np.float16)
        sbf_pad[pos] = sbf[tri].astype(np.float16)

        idx_grid = np.ascontiguousarray(kj_new.reshape(nchunk, P).T)
        ji_grid = np.ascontiguousarray(ji_sh.reshape(nchunk, P).T)
        # sbf^T single-stack [42, nchunk*P] (keeps all PE operands at base
        # partition 0 -- base-64 stationary + sub-bank PSUM dst faults).
        sbf_t = np.ascontiguousarray(
            sbf_pad.reshape(nchunk * P, SR).T)
        per_core.append(dict(idxg=idx_grid, jig=ji_grid, sbft=sbf_t))

    # globally renumbered x / rbf
    x_g = np.zeros((etot, HID), dtype=np.float32)
    x_g[new_global] = x
    rbf_g = np.zeros((etot, NR), dtype=np.float32)
    rbf_g[new_global] = rbf

    nsb_a = eperc // 512                     # phase-A chunks per core (49)
    ngrp_r = math.ceil(nsb_a / 3)
    for c in range(n_cores):
        sl = slice(c * eperc, (c + 1) * eperc)
        per_core[c]["xt16c"] = np.ascontiguousarray(
            x_g[sl].T.astype(np.float16))
        # rbf^T 3-stacked at partition offsets {0, 32, 64}
        rbf_c = np.zeros((ngrp_r * 3 * 512, NR), dtype=np.float32)
        rbf_c[:eperc] = rbf_g[sl]
        Rt = (rbf_c.reshape(ngrp_r, 3, 512, NR).transpose(1, 3, 0, 2)
              .reshape(3, NR, ngrp_r * 512).astype(np.float16))
        rbfp = np.zeros((64 + NR, ngrp_r * 512), dtype=np.float16)
        for g in range(3):
            rbfp[32 * g:32 * g + NR] = Rt[g]
        per_core[c]["rbfpc"] = rbfp

    dims = dict(n_cores=n_cores, E=E, T=T, eper=eper, nblk=nblk, eperc=eperc,
                etot=etot, K=K, cap=cap, nchunk=nchunk, nsb_a=nsb_a,
                ngrp_r=ngrp_r, nblkA=nblkA)
    shared = dict(
        iota_w=np.tile(np.arange(P, dtype=np.float16), (P, K)))
    return dims, shared, per_core, new_global


# ----------------------------------------------------------------------------
# Device program
# ----------------------------------------------------------------------------
def _build(nc, d):
    nblk, K, nchunk = d["nblk"], d["K"], d["nchunk"]
    eperc, nsb_a, ngrp_r = d["eperc"], d["nsb_a"], d["ngrp_r"]
    etot, nblkA = d["etot"], d["nblkA"]
    n_cores = d["n_cores"]
    nsb_c = nblk // 4
    TDT = F8 if TABLE_F8 else F16

    def din(name, shape, dt):
        return nc.dram_tensor(name, shape, dt, kind="ExternalInput").ap()

    xt16c = din("xt16c", [P, eperc], F16)
    rbfpc = din("rbfpc", [64 + NR, ngrp_r * 512], F16)
    iota_w = din("iota_w", [P, K * P], F16)
    sbft = din("sbft", [SR, nchunk * P], F16)
    idxg = din("idxg", [P, nchunk], I32)
    jig = din("jig", [P, nchunk], F16)

    # fp16 weights (host-cast); stacked rbf/sbf folded weights
    w16n = ["w_kj16", "w_down16", "w_ji16", "w_up16", "rb0_w116", "rb0_w216",
            "w_lin16", "ra0_w116", "ra0_w216", "ra1_w116", "ra1_w216"]
    wshape = dict(w_down16=[HID, INT], w_up16=[INT, HID])
    Wsb2 = din("Wsb2", [SR, INT], F16)
    Wr3 = din("Wr3", [64 + NR, HID], F16)
    W = {n: din(n, wshape.get(n, [HID, HID]), F16) for n in w16n}
    bn = ["b_kj", "b_ji", "b_lin", "rb0_b1", "rb0_b2", "ra0_b1", "ra0_b2",
          "ra1_b1", "ra1_b2"]
    B = {n: din(n, [P, 1], F32) for n in bn}

    shard = nc.dram_tensor("shard", [P, nblkA * INT], TDT).ap()
    table = nc.dram_tensor("table", [etot, INT], TDT).ap()
    outt = nc.dram_tensor("outt", [P, eperc], F16, kind="ExternalOutput").ap()

    Silu = ACT_FN
    MUL, ADD, EQ = (mybir.AluOpType.mult, mybir.AluOpType.add,
                    mybir.AluOpType.is_equal)

    with tile.TileContext(nc) as tc, ExitStack() as ctx:
        cp = ctx.enter_context(tc.tile_pool(name="const", bufs=1))

        wsb = {}
        for n in w16n:
            t = cp.tile(wshape.get(n, [HID, HID]), F16, tag=n)
            nc.sync.dma_start(out=t[:], in_=W[n][:, :])
            wsb[n] = t
        bsb = {}
        for n in bn:
            t = cp.tile([P, 1], F32, tag=f"b_{n}")
            nc.sync.dma_start(out=t[:], in_=B[n][:, :])
            bsb[n] = t
        iot = cp.tile([P, K * P], F16, tag="iota")
        nc.sync.dma_start(out=iot[:], in_=iota_w[:, :])
        idx_sb = cp.tile([P, nchunk], I32, tag="idxg")
        nc.sync.dma_start(out=idx_sb[:], in_=idxg[:, :])
        jig_sb = cp.tile([P, nchunk], F16, tag="jig")
        nc.sync.dma_start(out=jig_sb[:], in_=jig[:, :])
        wsb2 = cp.tile([SR, INT], F16, tag="Wsb2")
        nc.sync.dma_start(out=wsb2[:], in_=Wsb2[:, :])
        wr3 = cp.tile([64 + NR, HID], F16, tag="Wr3")
        nc.sync.dma_start(out=wr3[:], in_=Wr3[:, :])
        # resident x (feature-major fp16), used by phases A and C
        xsb = cp.tile([P, eperc], F16, tag="xsb")
        half = eperc // 2
        nc.sync.dma_start(out=xsb[:, :half], in_=xt16c[:, :half])
        nc.sync.dma_start(out=xsb[:, half:], in_=xt16c[:, half:])

        # ---------------- Phase A: sharded gather-table build ---------------
        tbfull = cp.tile([P, nblkA * INT], TDT, tag="tbfull")
        with tc.tile_pool(name="pa_sb", bufs=3) as pa, \
             tc.tile_pool(name="pa_ps", bufs=2, space="PSUM") as pap, \
             tc.tile_pool(name="pa_io", bufs=2) as pio:
            rt = None
            for s in range(nsb_a):
                if s % 3 == 0:
                    rt = pio.tile([64 + NR, 512], F16, tag="rt")
                    nc.sync.dma_start(
                        out=rt[:],
                        in_=rbfpc[:, (s // 3) * 512:(s // 3 + 1) * 512])
                ps1 = pap.tile([P, 512], F32, tag="ps1", space="PSUM")
                nc.tensor.matmul(ps1[:], wsb["w_kj16"][:],
                                 xsb[:, s * 512:(s + 1) * 512],
                                 start=True, stop=True)
                xkj = pa.tile([P, 512], F16, tag="xkj")
                nc.scalar.activation(xkj[:], ps1[:], Silu, bias=bsb["b_kj"][:])
                m = s % 3
                ps2 = pap.tile([P, 512], F32, tag="ps2", space="PSUM")
                nc.tensor.matmul(ps2[:], wr3[32 * m:32 * m + NR, :],
                                 rt[32 * m:32 * m + NR, :],
                                 start=True, stop=True)
                xkm = pa.tile([P, 512], F16, tag="xkm")
                nc.vector.tensor_tensor(out=xkm[:], in0=xkj[:], in1=ps2[:],
                                        op=MUL)
                pd = pap.tile([P, 4 * INT], F32, tag="pd", space="PSUM")
                for j in range(4):
                    nc.tensor.matmul(pd[:, j * INT:(j + 1) * INT],
                                     xkm[:, j * P:(j + 1) * P],
                                     wsb["w_down16"][:],
                                     start=True, stop=True)
                nc.scalar.activation(
                    tbfull[:, s * 4 * INT:(s + 1) * 4 * INT], pd[:], Silu)
            nc.sync.dma_start(out=shard[:, :], in_=tbfull[:])

        # ---------------- AllGather: replicate table shards ------------------
        nc.gpsimd.collective_compute(
            "AllGather", mybir.AluOpType.bypass,
            replica_groups=[list(range(n_cores))],
            ins=[shard[:, :].opt()], outs=[table[:, :].opt()])
        # CC cores run collectives in order; a barrier AllReduce whose
        # output is read back makes the table delivery observable.
        bar_in = nc.dram_tensor("bar_in", [P, 4], F32).ap()
        bar_out = nc.dram_tensor("bar_out", [P, 4], F32).ap()
        bar_sb = cp.tile([P, 4], F32, tag="bar_sb")
        nc.gpsimd.memset(bar_sb[:], 0.0)
        nc.gpsimd.dma_start(out=bar_in[:, :], in_=bar_sb[:])
        nc.gpsimd.collective_compute(
            "AllReduce", mybir.AluOpType.add,
            replica_groups=[list(range(n_cores))],
            ins=[bar_in[:, :].opt()], outs=[bar_out[:, :].opt()])
        bar_rd = cp.tile([P, 4], F32, tag="bar_rd")
        nc.gpsimd.dma_start(out=bar_rd[:], in_=bar_out[:, :])

        # ---------------- Phase B + C: gather/scatter + dense tail ----------
        with tc.tile_pool(name="pb_sb", bufs=3) as pb, \
             tc.tile_pool(name="pb_big", bufs=2) as pbig, \
             tc.tile_pool(name="pb_ps", bufs=2, space="PSUM") as pbp, \
             tc.tile_pool(name="pc_sb", bufs=2) as pc, \
             tc.tile_pool(name="pc_ps", bufs=2, space="PSUM") as pcp:
            csb = 4 * K  # chunks per superblock
            ngr = math.ceil(K / 4)
            aggs = None

            def mm(ps, w, rhs, W):
                # matmul moving-dim/PSUM-bank limit: emit in 512-col pieces
                for o in range(0, W, 512):
                    nc.tensor.matmul(ps[:, o:o + 512], w,
                                     rhs[:, o:o + 512], start=True, stop=True)

            def tail(cols, W):
                """dense per-edge tail over W edges (feature-major fp16)."""
                xl = xsb[:, cols]
                pup = pcp.tile([P, 1024], F32, tag="psC", space="PSUM")
                mm(pup, wsb["w_up16"][:], aggs, W)
                xup = pc.tile([P, 1024], F16, tag="xup")
                nc.scalar.activation(xup[:, :W], pup[:, :W], Silu)
                pji = pcp.tile([P, 1024], F32, tag="psC", space="PSUM")
                mm(pji, wsb["w_ji16"][:], xl, W)
                hji = pc.tile([P, 1024], F16, tag="hji")
                nc.scalar.activation(hji[:, :W], pji[:, :W], Silu,
                                     bias=bsb["b_ji"][:])
                h = pc.tile([P, 1024], F16, tag="h0")
                nc.vector.tensor_tensor(out=h[:, :W], in0=hji[:, :W],
                                        in1=xup[:, :W], op=ADD)

                def res(hin, w1, b1, w2, b2, tg):
                    p1 = pcp.tile([P, 1024], F32, tag="psC", space="PSUM")
                    mm(p1, wsb[w1][:], hin, W)
                    t1 = pc.tile([P, 1024], F16, tag=f"t1{tg}")
                    nc.scalar.activation(t1[:, :W], p1[:, :W], Silu,
                                         bias=bsb[b1][:])
                    p2 = pcp.tile([P, 1024], F32, tag="psC", space="PSUM")
                    mm(p2, wsb[w2][:], t1, W)
                    t2 = pc.tile([P, 1024], F16, tag=f"t2{tg}")
                    nc.scalar.activation(t2[:, :W], p2[:, :W], Silu,
                                         bias=bsb[b2][:])
                    ho = pc.tile([P, 1024], F16, tag=f"h{tg}")
                    nc.vector.tensor_tensor(out=ho[:, :W], in0=hin[:, :W],
                                            in1=t2[:, :W], op=ADD)
                    return ho

                h = res(h, "rb0_w116", "rb0_b1", "rb0_w216", "rb0_b2", "r0")
                pl = pcp.tile([P, 1024], F32, tag="psC", space="PSUM")
                mm(pl, wsb["w_lin16"][:], h, W)
                hl = pc.tile([P, 1024], F16, tag="hl")
                nc.scalar.activation(hl[:, :W], pl[:, :W], Silu,
                                     bias=bsb["b_lin"][:])
                h = pc.tile([P, 1024], F16, tag="h1")
                nc.vector.tensor_tensor(out=h[:, :W], in0=hl[:, :W], in1=xl,
                                        op=ADD)
                h = res(h, "ra0_w116", "ra0_b1", "ra0_w216", "ra0_b2", "a0")
                h = res(h, "ra1_w116", "ra1_b1", "ra1_w216", "ra1_b2", "a1")
                nc.sync.dma_start(out=outt[:, cols], in_=h[:, :W])

            for s in range(nsb_c):
                gt = pbig.tile([P, csb * INT], TDT, tag="gt")
                nc.gpsimd.indirect_dma_start(
                    out=gt[:], out_offset=None, in_=table[:, :],
                    in_offset=bass.IndirectOffsetOnAxis(
                        ap=idx_sb[:, s * csb:(s + 1) * csb], axis=0))
                st = pbig.tile([SR, csb * P], F16, tag="st")
                nc.sync.dma_start(out=st[:],
                                  in_=sbft[:, s * csb * P:(s + 1) * csb * P])
                if s % 2 == 0:
                    aggs = pc.tile([INT, 1024], F16, tag="aggs")
                for j in range(4):
                    pagg = pbp.tile([INT, P], F32, tag="pagg", space="PSUM")
                    ohj = pb.tile([P, K * P], F16, tag="oh")
                    chj = s * csb + j * K
                    nc.vector.tensor_tensor(
                        out=ohj[:].rearrange("p (c w) -> p c w", w=P),
                        in0=iot[:].rearrange("p (c w) -> p c w", w=P),
                        in1=jig_sb[:, chj:chj + K].broadcast_to([P, K, P]),
                        op=EQ)
                    for kg in range(ngr):
                        g = min(4, K - 4 * kg)
                        k0 = 4 * kg
                        psE = pbp.tile([P, 4 * INT], F32, tag="psE",
                                       space="PSUM")
                        for q in range(g):
                            cc = j * K + k0 + q
                            nc.tensor.matmul(
                                psE[:, q * INT:(q + 1) * INT],
                                st[:, cc * P:(cc + 1) * P],
                                wsb2[:],
                                start=True, stop=True)
                        msg4 = pb.tile([P, 4 * INT], F16, tag="msg")
                        gt0 = (j * K + k0) * INT
                        nc.vector.tensor_tensor(
                            out=msg4[:, :g * INT],
                            in0=gt[:, gt0:gt0 + g * INT],
                            in1=psE[:, :g * INT], op=MUL)
                        for q in range(g):
                            nc.tensor.matmul(
                                pagg[:], msg4[:, q * INT:(q + 1) * INT],
                                ohj[:, (k0 + q) * P:(k0 + q + 1) * P],
                                start=(kg == 0 and q == 0),
                                stop=(k0 + q == K - 1))
                    nc.vector.tensor_copy(
                        out=aggs[:, (s % 2) * 512 + j * P:
                                 (s % 2) * 512 + (j + 1) * P],
                        in_=pagg[:])
                if s % 2 == 1:
                    tail(slice((s - 1) * 512, (s + 1) * 512), 1024)
                elif s == nsb_c - 1:
                    tail(slice(s * 512, (s + 1) * 512), 512)
    return outt


# ----------------------------------------------------------------------------
def _run(inputs, n_cores=8, trace=False):
    x = np.asarray(inputs["x"], np.float32)
    rbf = np.asarray(inputs["rbf"], np.float32)
    sbf = np.asarray(inputs["sbf"], np.float32)
    idx_kj = np.asarray(inputs["idx_kj"])
    idx_ji = np.asarray(inputs["idx_ji"])

    d, shared, per_core, new_global = _preprocess(
        x, rbf, sbf, idx_kj, idx_ji, n_cores)

    nc = bacc.Bacc("TRN2", target_bir_lowering=False, debug=False,
                   enable_asserts=False, num_devices=n_cores)
    _build(nc, d)
    nc.compile()

    f32, f16 = np.float32, np.float16
    base = dict(shared)
    wmap = dict(w_kj16="w_kj", w_down16="w_down", w_ji16="w_ji",
                w_up16="w_up", rb0_w116="rb0_w1", rb0_w216="rb0_w2",
                w_lin16="w_lin", ra0_w116="ra0_w1", ra0_w216="ra0_w2",
                ra1_w116="ra1_w1", ra1_w216="ra1_w2")
    for k16, k in wmap.items():
        base[k16] = np.ascontiguousarray(np.asarray(inputs[k], f32), f32
                                         ).astype(f16)
    wsb_f = (np.asarray(inputs["w_sbf1"], f32) @
             np.asarray(inputs["w_sbf2"], f32)).astype(f16)
    base["Wsb2"] = np.ascontiguousarray(wsb_f)
    wr_f = (np.asarray(inputs["w_rbf1"], f32) @
            np.asarray(inputs["w_rbf2"], f32)).astype(f16)
    Wr3 = np.zeros((64 + NR, HID), dtype=f16)
    for g in range(3):
        Wr3[32 * g:32 * g + NR] = wr_f
    base["Wr3"] = Wr3
    for bnm in ["b_kj", "b_ji", "b_lin", "rb0_b1", "rb0_b2", "ra0_b1",
                "ra0_b2", "ra1_b1", "ra1_b2"]:
        base[bnm] = np.ascontiguousarray(
            np.asarray(inputs[bnm], f32).reshape(P, 1))

    in_maps = []
    for c in range(n_cores):
        m = dict(base)
        m.update(per_core[c])
        in_maps.append(m)

    res = run_bass_kernel_spmd(nc, in_maps, core_ids=list(range(n_cores)),
                               trace=trace)
    h_full = np.concatenate([res.results[c]["outt"].T for c in range(n_cores)],
                            axis=0)
    out = h_full[new_global].astype(np.float32)
    return out, res


def kernel(**inputs):
    out, _ = _run(inputs, n_cores=8, trace=False)
    return out
